# revision 2
# baseline (speedup 1.0000x reference)
"""Trainium2 Bass kernel for BiAttention (b=8, n=m=1024, d=512).

Sharding: data-parallel over batch — one batch element per NeuronCore,
8 cores, no cross-core communication.

Per-core algorithm. Softmax shift-invariance folds the Linear(3d,1)
row/col terms, the bias and the padding masks into per-row/col exponent
weights g1[n] = exp(s1[n])*valid1[n], g2[m] = exp(s2[m])*valid2[m]
(logits ~N(0,1) so raw exp is safe; masked rows get weight exactly 0):

  E[n,m]   = exp((x1*w3) @ x2^T)        raw, ungated   (n-part, m-free)
  ET       = E^T via PE transposes      raw             (m-part, n-free)
  den2[m]  = sum_n g1[n] E[n,m]         M=1 row matmul, lhsT=g1 column
  den1[n]  = sum_m g2[m] ET[m,n]        M=1 row matmul, lhsT=g2 column
  q2c      = (E^T @ (g1*x1)) / den2     g1 folded into the rhs copy
  c2q      = (ET^T @ (g2*x2)) / den1
  q2c_att  = (ET^T @ (q2c*g2*rden2_dup... )) / den1
  out      = [x1, c2q, x1*c2q, x1*q2c_att]

All gating lives in cheap per-partition column scalings (rhs copies and
eviction scales), so E/ET are evicted as plain exp/copy — no separate
gating passes over the big (n,m) matrix.  s1 rides as an appended
u1=w1/w3 column on the last E matmul chunk (exp on eviction gives
exp(s1) directly); s2 is one M=1 row-matmul pass over x2T.  W and the
masks are DMA'd in [rows,128] layout so each becomes a single batched
PE transpose instead of per-column transposes.  f32<->f32r uses
AP.bitcast (no staging copies); the bf16 variant converts x1/x2 once so
all transposes run at bf16 LDWEIGHTS/stream rates.

Mask-suffix specialization: tiles of 128 that are fully masked at the
end of either sequence are skipped in the contractions; the host
dispatches to a NEFF compiled for (kn, km) kept-tile counts.
Partially-masked tiles are exact via the 0/1 valid columns inside
g1/g2.
"""

import numpy as np
from contextlib import ExitStack

import concourse.bacc as bacc
import concourse.tile as tile
import concourse.mybir as mybir
from concourse.bass_utils import run_bass_kernel_spmd
from concourse.masks import make_identity

F32 = mybir.dt.float32
U8 = mybir.dt.uint8
R = mybir.dt.float32r
BF = mybir.dt.bfloat16
EXP = mybir.ActivationFunctionType.Exp
COPY = mybir.ActivationFunctionType.Copy
MULT = mybir.AluOpType.mult
ADD = mybir.AluOpType.add

P = 128
N = 1024          # x1 rows
M = 1024          # x2 rows
D = 512           # feature dim
NT, MT, DC = N // P, M // P, D // P

N_CORES = 8
MM_DTYPE = BF     # bf16: fast transposes; f32r fallback is exact

_CACHE = {}


def _chunks512(width):
    out, o = [], 0
    while o < width:
        w = min(512, width - o)
        out.append((o, w))
        o += w
    return out


def _chunks_e(width):
    """Chunks for the E matmul: last chunk <= 511 so the +1 s1 column fits."""
    out = _chunks512(width)
    if out[-1][1] == 512:
        o, _ = out[-1]
        out[-1] = (o, 256)
        out.append((o + 256, 256))
    return out


def _build(kn, km, mm=MM_DTYPE):
    bf = (mm == BF)
    vm = km * P
    nc = bacc.Bacc("TRN2", target_bir_lowering=False, debug=False)
    x1d = nc.dram_tensor("x1", [N, D], F32, kind="ExternalInput").ap()
    x2d = nc.dram_tensor("x2", [M, D], F32, kind="ExternalInput").ap()
    m1d = nc.dram_tensor("x1_mask", [N], U8, kind="ExternalInput").ap()
    m2d = nc.dram_tensor("x2_mask", [M], U8, kind="ExternalInput").ap()
    wd = nc.dram_tensor("W", [3 * D], F32, kind="ExternalInput").ap()
    outd = nc.dram_tensor("out", [N, 4 * D], F32, kind="ExternalOutput").ap()

    x1r_d = x1d.rearrange("(t p) d -> p t d", p=P)
    x2r_d = x2d.rearrange("(t p) d -> p t d", p=P)
    out_r = outd.rearrange("(t p) e -> p t e", p=P)

    ech = _chunks_e(vm)          # E matmul chunks (last takes +1 u1 col)
    rch = _chunks512(vm)         # row-matmul chunks over m
    nch = _chunks512(N)          # row-matmul chunks over n

    with tile.TileContext(nc) as tc, ExitStack() as ctx:
        const = ctx.enter_context(tc.tile_pool(name="const", bufs=1))
        big = ctx.enter_context(tc.tile_pool(name="big", bufs=1))
        rows = ctx.enter_context(tc.tile_pool(name="rows", bufs=1))
        work = ctx.enter_context(tc.tile_pool(name="work", bufs=3))
        psE = ctx.enter_context(tc.tile_pool(name="psE", bufs=3, space="PSUM"))
        psT = ctx.enter_context(tc.tile_pool(name="psT", bufs=2, space="PSUM"))
        psR = ctx.enter_context(tc.tile_pool(name="psR", bufs=1, space="PSUM"))
        psC = ctx.enter_context(tc.tile_pool(name="psC", bufs=2, space="PSUM"))

        # ---------------- loads (issue order = priority) ----------------
        wsb = rows.tile([12, P], F32)
        nc.sync.dma_start(wsb[:], wd.rearrange("(c p) -> c p", p=P))
        m1s = rows.tile([NT, P], U8)
        nc.sync.dma_start(m1s[:], m1d.rearrange("(t p) -> t p", p=P))
        m2s = rows.tile([MT, P], U8)
        nc.sync.dma_start(m2s[0:km, :], m2d.rearrange("(t p) -> t p", p=P)[0:km, :])

        x1n = big.tile([P, NT, D], F32)
        x2n = big.tile([P, MT, D], F32)
        nc.sync.dma_start(x1n[:, 0:4, :], x1r_d[:, 0:4, :])
        kml = min(km, 4)
        nc.sync.dma_start(x2n[:, 0:kml, :], x2r_d[:, 0:kml, :])
        if km > 4:
            nc.sync.dma_start(x2n[:, 4:km, :], x2r_d[:, 4:km, :])
        nc.sync.dma_start(x1n[:, 4:8, :], x1r_d[:, 4:8, :])

        # ---------------- constants / prep ----------------
        ident = const.tile([P, P], F32)
        make_identity(nc, ident)
        if bf:
            identm = const.tile([P, P], BF)
            nc.vector.tensor_copy(identm[:], ident[:])
        else:
            identm = ident.bitcast(R)

        # W columns: one batched transpose [12,128] -> [128,12]
        pw = psC.tile([P, 12], F32, tag="ps_col")
        nc.tensor.transpose(pw[:], wsb[0:12, :], ident[0:12, 0:12])
        wcols = const.tile([P, 12], F32)   # w1=0:4 w2=4:8 w3=8:12
        nc.vector.tensor_copy(wcols[:], pw[:])
        w3rec = const.tile([P, 4], F32)
        nc.vector.reciprocal(w3rec[:], wcols[:, 8:12])
        u1f = const.tile([P, 4], F32)      # w1/w3: recovers s1 from x1w3T
        nc.vector.tensor_mul(u1f[:], wcols[:, 0:4], w3rec[:])
        if bf:
            w2m = const.tile([P, 4], BF)
            nc.vector.tensor_copy(w2m[:], wcols[:, 4:8])
        else:
            w2m = wcols[:, 4:8].bitcast(R)

        # masks -> 0/1 valid columns
        m1f = rows.tile([NT, P], F32)
        nc.vector.tensor_copy(m1f[:], m1s[:])
        pm1 = psC.tile([P, NT], F32, tag="ps_col")
        nc.tensor.transpose(pm1[:], m1f[:], ident[0:NT, 0:NT])
        valid1 = const.tile([P, NT], F32)
        nc.vector.tensor_scalar(valid1[:], pm1[:], -1.0, 1.0, MULT, ADD)
        m2f = rows.tile([MT, P], F32)
        nc.vector.tensor_copy(m2f[0:km, :], m2s[0:km, :])
        pm2 = psC.tile([P, km], F32, tag="ps_col")
        nc.tensor.transpose(pm2[:], m2f[0:km, :], ident[0:km, 0:km])
        valid2 = const.tile([P, MT], F32)
        nc.vector.tensor_scalar(valid2[:, 0:km], pm2[:], -1.0, 1.0, MULT, ADD)

        # ---------------- transposed operands ----------------
        if bf:
            x1b = big.tile([P, NT, D], BF)
            x2b = big.tile([P, MT, D], BF)
        else:
            x1b = x1n.bitcast(R)
            x2b = x2n.bitcast(R)
        x1w3T = big.tile([P, DC, N], mm)       # (d-chunk, n) of x1*w3
        x2T = big.tile([P, DC, vm + 1], mm)    # (d-chunk, m) of x2; col vm = u1
        nc.vector.tensor_copy(x2T[:, :, vm:vm + 1], u1f.unsqueeze(2))

        def x1_quad(q):
            if bf:
                for j in range(4):
                    nc.scalar.copy(x1b[:, q * 4 + j, :], x1n[:, q * 4 + j, :])
            for c in range(DC):
                pq = psT.tile([P, 512], mm, tag="ps_tr", name=f"x1q_{q}_{c}")
                for j in range(4):
                    nc.tensor.transpose(pq[:, j * P:(j + 1) * P],
                                        x1b[:, q * 4 + j, c * P:(c + 1) * P],
                                        identm[:])
                # evict fused with w3 scaling (per-partition in (d, n) layout)
                nc.vector.tensor_scalar_mul(
                    x1w3T[:, c, q * 512:(q + 1) * 512], pq[:],
                    wcols[:, 8 + c:9 + c])

        def x2_quad(q):
            jw = min(4, km - q * 4)
            if bf:
                for j in range(jw):
                    nc.scalar.copy(x2b[:, q * 4 + j, :], x2n[:, q * 4 + j, :])
            for c in range(DC):
                pq = psT.tile([P, 512], mm, tag="ps_tr", name=f"x2q_{q}_{c}")
                for j in range(jw):
                    nc.tensor.transpose(pq[:, j * P:(j + 1) * P],
                                        x2b[:, q * 4 + j, c * P:(c + 1) * P],
                                        identm[:])
                nc.scalar.copy(x2T[:, c, q * 512:q * 512 + jw * P],
                               pq[:, 0:jw * P])

        x1_quad(0)
        x2_quad(0)
        if km > 4:
            x2_quad(1)

        # s2 row: M=1 matmuls over x2T, then one batched set of col transposes
        s2row = rows.tile([1, vm], F32)
        for h, (off, w) in enumerate(rch):
            pr = psR.tile([1, 512], F32, tag="ps_row", name=f"s2r_{h}")
            for c in range(DC):
                nc.tensor.matmul(pr[0:1, 0:w], w2m[:, c:c + 1],
                                 x2T[:, c, off:off + w],
                                 start=(c == 0), stop=(c == DC - 1))
            nc.vector.tensor_copy(s2row[:, off:off + w], pr[0:1, 0:w])
        ps2 = psC.tile([P, km], F32, tag="ps_col")
        for u in range(km):
            nc.tensor.transpose(ps2[:, u:u + 1], s2row[0:1, u * P:(u + 1) * P],
                                ident[0:1, 0:1])
        es2 = const.tile([P, MT], F32)
        nc.scalar.activation(es2[:, 0:km], ps2[:], EXP)
        g2c = const.tile([P, MT], F32)
        nc.vector.tensor_mul(g2c[:, 0:km], es2[:, 0:km], valid2[:, 0:km])
        if bf:
            g2m = const.tile([P, MT], BF)
            nc.vector.tensor_copy(g2m[:, 0:km], g2c[:, 0:km])
        else:
            g2m = g2c.bitcast(R)

        # g2-weighted x2 (rhs of U_row)
        x2g = big.tile([P, MT, D], mm)
        for u in range(km):
            nc.vector.tensor_scalar_mul(x2g[:, u, :], x2b[:, u, :],
                                        g2c[:, u:u + 1])

        # ---------------- E = exp(tri) ----------------
        E = big.tile([P, NT, vm], mm)
        es1 = const.tile([P, NT], F32)
        last_h = len(ech) - 1

        def e_tile(t):
            for h, (off, w) in enumerate(ech):
                wid = w + (1 if h == last_h else 0)   # s1 column rides last
                pe = psE.tile([P, 512], F32, tag="ps_big", name=f"pe_{t}_{h}")
                for c in range(DC):
                    nc.tensor.matmul(pe[:, 0:wid],
                                     x1w3T[:, c, t * P:(t + 1) * P],
                                     x2T[:, c, off:off + wid],
                                     start=(c == 0), stop=(c == DC - 1))
                nc.scalar.activation(E[:, t, off:off + w], pe[:, 0:w], EXP)
                if h == last_h:
                    nc.scalar.activation(es1[:, t:t + 1], pe[:, w:w + 1], EXP)

        for t in range(4):
            e_tile(t)
        x1_quad(1)
        nc.sync.dma_start(out_r[:, 0:4, 0:D], x1n[:, 0:4, :])   # block 0 lo
        for t in range(4, 8):
            e_tile(t)

        # g1 column + g1-weighted x1 (rhs of U_col)
        g1c = const.tile([P, NT], F32)
        nc.vector.tensor_mul(g1c[:], es1[:], valid1[:])
        if bf:
            g1m = const.tile([P, NT], BF)
            nc.vector.tensor_copy(g1m[:], g1c[:])
        else:
            g1m = g1c.bitcast(R)
        x1g = big.tile([P, NT, D], mm)
        for k in range(kn):
            nc.vector.tensor_scalar_mul(x1g[:, k, :], x1b[:, k, :],
                                        g1c[:, k:k + 1])

        # ---------------- ET = E^T (raw transposes) ----------------
        ET = big.tile([P, MT, N], mm)

        def et_quad(u, tq):
            pq = psT.tile([P, 512], mm, tag="ps_tr", name=f"eq_{u}_{tq}")
            for j in range(4):
                nc.tensor.transpose(pq[:, j * P:(j + 1) * P],
                                    E[:, tq * 4 + j, u * P:(u + 1) * P],
                                    identm[:])
            if u % 2 == 0:
                nc.scalar.copy(ET[:, u, tq * 512:(tq + 1) * 512], pq[:])
            else:
                nc.vector.tensor_copy(ET[:, u, tq * 512:(tq + 1) * 512], pq[:])

        for u in range(km):
            et_quad(u, 0)
        nc.sync.dma_start(out_r[:, 4:8, 0:D], x1n[:, 4:8, :])   # block 0 hi

        # den2[m] = sum_n g1[n] E[n,m]  (row matmuls, lhsT = g1 column)
        d2row = rows.tile([1, vm], F32)
        for h, (off, w) in enumerate(rch):
            pr = psR.tile([1, 512], F32, tag="ps_row", name=f"d2r_{h}")
            for k in range(kn):
                nc.tensor.matmul(pr[0:1, 0:w], g1m[:, k:k + 1],
                                 E[:, k, off:off + w],
                                 start=(k == 0), stop=(k == kn - 1))
            nc.vector.tensor_copy(d2row[:, off:off + w], pr[0:1, 0:w])

        for u in range(km):
            et_quad(u, 1)

        pd2 = psC.tile([P, km], F32, tag="ps_col")
        for u in range(km):
            nc.tensor.transpose(pd2[:, u:u + 1], d2row[0:1, u * P:(u + 1) * P],
                                ident[0:1, 0:1])
        rden2 = const.tile([P, MT], F32)
        nc.vector.reciprocal(rden2[:, 0:km], pd2[:])
        qscale = const.tile([P, MT], F32)     # rden2 * g2: Q2C pre-gated for V
        nc.vector.tensor_mul(qscale[:, 0:km], rden2[:, 0:km], g2c[:, 0:km])

        # den1[n] = sum_m g2[m] ET[m,n]
        d1row = rows.tile([1, N], F32)
        for h, (off, w) in enumerate(nch):
            pr = psR.tile([1, 512], F32, tag="ps_row", name=f"d1r_{h}")
            for k in range(km):
                nc.tensor.matmul(pr[0:1, 0:w], g2m[:, k:k + 1],
                                 ET[:, k, off:off + w],
                                 start=(k == 0), stop=(k == km - 1))
            nc.vector.tensor_copy(d1row[:, off:off + w], pr[0:1, 0:w])
        pd1 = psC.tile([P, NT], F32, tag="ps_col")
        for t in range(NT):
            nc.tensor.transpose(pd1[:, t:t + 1], d1row[0:1, t * P:(t + 1) * P],
                                ident[0:1, 0:1])
        rden1 = const.tile([P, NT], F32)
        nc.vector.reciprocal(rden1[:], pd1[:])

        # x1 * rden1 — single-op block2/block3 finals straight from PSUM
        x1r1 = big.tile([P, NT, D], F32)
        for t in range(NT):
            nc.vector.tensor_scalar_mul(x1r1[:, t, :], x1n[:, t, :],
                                        rden1[:, t:t + 1])

        # ---------------- U_col -> Q2C (pre-scaled by rden2*g2) ----------------
        Q2C = big.tile([P, MT, D], mm)
        for u in range(km):
            pu = psE.tile([P, 512], F32, tag="ps_big", name=f"pu_{u}")
            for k in range(kn):
                nc.tensor.matmul(pu[:], E[:, k, u * P:(u + 1) * P],
                                 x1g[:, k, :],
                                 start=(k == 0), stop=(k == kn - 1))
            nc.scalar.activation(Q2C[:, u, :], pu[:], COPY,
                                 scale=qscale[:, u:u + 1])

        # ---------------- U_row -> c2q; out blocks 1, 2 ----------------
        for t in range(NT):
            pr = psE.tile([P, 512], F32, tag="ps_big", name=f"pr_{t}")
            for k in range(km):
                nc.tensor.matmul(pr[:], ET[:, k, t * P:(t + 1) * P],
                                 x2g[:, k, :],
                                 start=(k == 0), stop=(k == km - 1))
            combo = work.tile([P, 2 * D], F32, tag="ev", name=f"cb_{t}")
            nc.scalar.activation(combo[:, 0:D], pr[:], COPY,
                                 scale=rden1[:, t:t + 1])
            nc.vector.tensor_mul(combo[:, D:2 * D], x1r1[:, t, :], pr[:])
            nc.sync.dma_start(out_r[:, t, D:3 * D], combo[:])

        # ---------------- V; out block 3 = x1*rden1*V ----------------
        for t in range(NT):
            pv = psE.tile([P, 512], F32, tag="ps_big", name=f"pv_{t}")
            for k in range(km):
                nc.tensor.matmul(pv[:], ET[:, k, t * P:(t + 1) * P],
                                 Q2C[:, k, :],
                                 start=(k == 0), stop=(k == km - 1))
            b3 = work.tile([P, D], F32, tag="ev3", name=f"b3_{t}")
            nc.vector.tensor_mul(b3[:], x1r1[:, t, :], pv[:])
            nc.sync.dma_start(out_r[:, t, 3 * D:4 * D], b3[:])

    nc.compile()
    return nc


def _kept_tiles(mask):
    """Tiles (of 128) up to and including the last one with any valid row."""
    valid = ~mask.astype(bool)           # (b, L)
    any_valid = valid.reshape(valid.shape[0], -1, P).any(axis=2).any(axis=0)
    nz = np.nonzero(any_valid)[0]
    return int(nz[-1]) + 1 if len(nz) else 1


def _get_nc(kn, km):
    key = (kn, km)
    if key not in _CACHE:
        _CACHE[key] = _build(kn, km)
    return _CACHE[key]


def _run(inputs, trace=False, trace_cores=None):
    x1 = np.ascontiguousarray(np.asarray(inputs["x1"], dtype=np.float32))
    x2 = np.ascontiguousarray(np.asarray(inputs["x2"], dtype=np.float32))
    m1 = np.ascontiguousarray(np.asarray(inputs["x1_mask"]).astype(np.uint8))
    m2 = np.ascontiguousarray(np.asarray(inputs["x2_mask"]).astype(np.uint8))
    W = np.ascontiguousarray(np.asarray(inputs["W"], dtype=np.float32))
    nc = _get_nc(_kept_tiles(m1), _kept_tiles(m2))
    in_maps = [
        {"x1": x1[i], "x2": x2[i], "x1_mask": m1[i], "x2_mask": m2[i], "W": W}
        for i in range(N_CORES)
    ]
    res = run_bass_kernel_spmd(nc, in_maps, core_ids=list(range(N_CORES)),
                               trace=trace, trace_cores=trace_cores)
    out = np.stack([res.results[i]["out"] for i in range(N_CORES)], axis=0)
    return out.astype(np.float32), res


def kernel(x1, x1_mask, x2, x2_mask, W, bias=None, **_kw):
    # bias is mathematically irrelevant: a global additive constant cancels in
    # both softmaxes, and every output term is softmax-weighted.
    out, _ = _run({"x1": x1, "x1_mask": x1_mask, "x2": x2, "x2_mask": x2_mask,
                   "W": W})
    return out


# revision 12
# speedup vs baseline: 1.2645x; 1.2645x over previous
"""Trainium2 Bass kernel for BiAttention (b=8, n=m=1024, d=512).

Sharding: data-parallel over batch — one batch element per NeuronCore,
8 cores, no cross-core communication.

Per-core algorithm. Softmax shift-invariance folds the Linear(3d,1)
row/col terms, the bias and the padding masks into per-row/col exponent
weights g1[n] = exp(s1[n])*valid1[n], g2[m] = exp(s2[m])*valid2[m]
(logits ~N(0,1) so raw exp is safe; masked rows get weight exactly 0):

  E[n,m]   = exp((x1*w3) @ x2^T - 3)    raw, ungated   (n-part, m-free)
  ET       = E^T via PE transposes      raw             (m-part, n-free)
  den2[m]  = sum_n g1[n] E[n,m]         M=1 row matmul, lhsT=g1 column
  den1[n]  = sum_m g2[m] ET[m,n]        M=1 row matmul, lhsT=g2 column
  q2c      = (E^T @ (g1*x1)) / den2
  c2q      = (ET^T @ (g2*x2)) / den1
  q2c_att  = (ET^T @ (q2c*g2*rden2)) / den1
  out      = [x1, c2q, x1*c2q, x1*q2c_att]

The -3 exponent shift centers exp values in fp8e4m3 range (TRN e4m3
saturates to Inf at 256); every consumer is a num/den ratio so the
shift cancels exactly.  All gating lives in per-partition column
scalings (rhs copies and eviction scales), so E/ET evict as plain
exp/copy.  The big contractions run fp8e4m3 with DoubleRow (2 k-tiles
per matmul, 2x PE throughput); odd tail tiles fall back to plain fp8
matmuls.  x1/x2 transpose directly from their f32 DMA tiles (the
eviction converts to fp8), so nothing gates the pipeline front.  Inputs
load as per-tile DMAs to spread across the 16 DMA queues.  U_row and V
interleave per tile so the block finals spread instead of piling into a
tail; block2 runs on the otherwise-idle GpSimd engine.

Mask-suffix specialization: tiles of 128 that are fully masked at the
end of either sequence are skipped in the contractions; the host
dispatches to a NEFF compiled for (kn, km) kept-tile counts.
Partially-masked tiles are exact via the 0/1 valid columns inside
g1/g2.
"""

import numpy as np
from contextlib import ExitStack

import concourse.bacc as bacc
import concourse.tile as tile
import concourse.mybir as mybir
from concourse.bass_utils import run_bass_kernel_spmd
from concourse.masks import make_identity

F32 = mybir.dt.float32
U8 = mybir.dt.uint8
R = mybir.dt.float32r
BF = mybir.dt.bfloat16
FP8 = mybir.dt.float8e4
DR = mybir.MatmulPerfMode.DoubleRow
EXP = mybir.ActivationFunctionType.Exp
COPY = mybir.ActivationFunctionType.Copy
MULT = mybir.AluOpType.mult
ADD = mybir.AluOpType.add

P = 128
N = 1024          # x1 rows
M = 1024          # x2 rows
D = 512           # feature dim
NT, MT, DC = N // P, M // P, D // P
ESHIFT = -3.0     # exp(tri + ESHIFT): centers E in fp8e4m3 range; cancels

N_CORES = 8
MM_DTYPE = FP8    # fp8e4m3 + DoubleRow; BF fallback is plain-rate

_CACHE = {}


def _chunks512(width):
    out, o = [], 0
    while o < width:
        w = min(512, width - o)
        out.append((o, w))
        o += w
    return out


def _chunks_e(width):
    """Chunks for the E matmul: last chunk <= 511 so the +1 s1 column fits."""
    out = _chunks512(width)
    if out[-1][1] == 512:
        o, _ = out[-1]
        out[-1] = (o, 256)
        out.append((o + 256, 256))
    return out


def _pairs(k):
    """[(idx, 2), ...] DoubleRow pairs plus a trailing single if k is odd."""
    out = [(i, 2) for i in range(0, k - 1, 2)]
    if k % 2:
        out.append((k - 1, 1))
    return out


def _build(kn, km, mm=MM_DTYPE):
    fp8 = (mm == FP8)
    vm = km * P
    vstride = ((vm + 1 + 15) // 16) * 16   # x2T free stride, 16B-aligned for DR
    nc = bacc.Bacc("TRN2", target_bir_lowering=False, debug=False)
    x1d = nc.dram_tensor("x1", [N, D], F32, kind="ExternalInput").ap()
    x2d = nc.dram_tensor("x2", [M, D], F32, kind="ExternalInput").ap()
    m1d = nc.dram_tensor("x1_mask", [N], U8, kind="ExternalInput").ap()
    m2d = nc.dram_tensor("x2_mask", [M], U8, kind="ExternalInput").ap()
    wd = nc.dram_tensor("W", [3 * D], F32, kind="ExternalInput").ap()
    outd = nc.dram_tensor("out", [N, 4 * D], F32, kind="ExternalOutput").ap()

    x1r_d = x1d.rearrange("(t p) d -> p t d", p=P)
    x2r_d = x2d.rearrange("(t p) d -> p t d", p=P)
    out_r = outd.rearrange("(t p) e -> p t e", p=P)

    ech = _chunks_e(vm)          # E matmul chunks (last takes +1 u1 col)
    rch = _chunks512(vm)         # row-matmul chunks over m
    nch = _chunks512(N)          # row-matmul chunks over n

    def mm_pairs(k):
        return _pairs(k) if fp8 else [(i, 1) for i in range(k)]

    with tile.TileContext(nc) as tc, ExitStack() as ctx:
        const = ctx.enter_context(tc.tile_pool(name="const", bufs=1))
        big = ctx.enter_context(tc.tile_pool(name="big", bufs=1))
        rows = ctx.enter_context(tc.tile_pool(name="rows", bufs=1))
        work = ctx.enter_context(tc.tile_pool(name="work", bufs=3))
        psA = ctx.enter_context(tc.tile_pool(name="psA", bufs=3, space="PSUM"))
        psT = ctx.enter_context(tc.tile_pool(name="psT", bufs=2, space="PSUM"))
        psR = ctx.enter_context(tc.tile_pool(name="psR", bufs=1, space="PSUM"))
        psC = ctx.enter_context(tc.tile_pool(name="psC", bufs=2, space="PSUM"))

        # ------------- loads: per-tile DMAs spread across queues -------------
        wsb = rows.tile([12, P], F32)
        nc.sync.dma_start(wsb[:], wd.rearrange("(c p) -> c p", p=P))
        m1s = rows.tile([NT, P], U8)
        nc.sync.dma_start(m1s[:], m1d.rearrange("(t p) -> t p", p=P))
        m2s = rows.tile([MT, P], U8)
        nc.sync.dma_start(m2s[0:km, :], m2d.rearrange("(t p) -> t p", p=P)[0:km, :])

        x1n = big.tile([P, NT, D], F32)
        x2n = big.tile([P, MT, D], F32)
        for t in range(4):
            nc.sync.dma_start(x1n[:, t, :], x1r_d[:, t, :])
        for u in range(km):
            nc.sync.dma_start(x2n[:, u, :], x2r_d[:, u, :])
        for t in range(4, 8):
            nc.sync.dma_start(x1n[:, t, :], x1r_d[:, t, :])

        # ---------------- constants ----------------
        ident = const.tile([P, P], F32)
        make_identity(nc, ident)
        identm = const.tile([P, P], mm)
        nc.vector.tensor_copy(identm[:], ident[:])
        ebias = const.tile([P, 1], F32)
        nc.vector.memset(ebias[:], ESHIFT)

        # W columns: one batched transpose [12,128] -> [128,12]
        pw = psC.tile([P, 12], F32, tag="ps_col")
        nc.tensor.transpose(pw[:], wsb[0:12, :], ident[0:12, 0:12])
        wcols = const.tile([P, 12], F32)   # w1=0:4 w2=4:8 w3=8:12
        nc.vector.tensor_copy(wcols[:], pw[:])
        w3rec = const.tile([P, 4], F32)
        nc.vector.reciprocal(w3rec[:], wcols[:, 8:12])
        u1f = const.tile([P, 4], F32)      # w1/w3: recovers s1 from x1w3T
        nc.vector.tensor_mul(u1f[:], wcols[:, 0:4], w3rec[:])
        w2m = const.tile([P, 4], mm)
        nc.vector.tensor_copy(w2m[:], wcols[:, 4:8])
        # u1 spans +-202; fp8 PE weights break above ~16 (e6m3 upcast), so
        # store u1/16 and recover the factor in the exp-activation scale
        u1m = const.tile([P, 4], mm)
        nc.vector.tensor_scalar_mul(u1m[:], u1f[:], 1.0 / 16.0)

        # masks -> 0/1 valid columns
        m1f = rows.tile([NT, P], F32)
        nc.vector.tensor_copy(m1f[:], m1s[:])
        pm1 = psC.tile([P, NT], F32, tag="ps_col")
        nc.tensor.transpose(pm1[:], m1f[:], ident[0:NT, 0:NT])
        valid1 = const.tile([P, NT], F32)
        nc.vector.tensor_scalar(valid1[:], pm1[:], -1.0, 1.0, MULT, ADD)
        m2f = rows.tile([MT, P], F32)
        nc.vector.tensor_copy(m2f[0:km, :], m2s[0:km, :])
        pm2 = psC.tile([P, km], F32, tag="ps_col")
        nc.tensor.transpose(pm2[:], m2f[0:km, :], ident[0:km, 0:km])
        valid2 = const.tile([P, MT], F32)
        nc.vector.tensor_scalar(valid2[:, 0:km], pm2[:], -1.0, 1.0, MULT, ADD)

        # ---------------- transposed operands (f32 in, mm out) ----------------
        x1w3T = big.tile([P, DC, N], mm)        # (d-chunk, n) of x1*w3
        x2T = big.tile([P, DC, vstride], mm)    # (d-chunk, m) of x2
        nc.vector.memset(x2T[:, :, vm:vstride], 0.0)

        def x1_quad(q):
            for c in range(DC):
                pq = psA.tile([P, 512], F32, tag="ps_big", name=f"x1q_{q}_{c}")
                for j in range(4):
                    nc.tensor.transpose(pq[:, j * P:(j + 1) * P],
                                        x1n[:, q * 4 + j, c * P:(c + 1) * P],
                                        ident[:])
                # evict fused with w3 scaling (per-partition in (d, n) layout)
                nc.vector.tensor_scalar_mul(
                    x1w3T[:, c, q * 512:(q + 1) * 512], pq[:],
                    wcols[:, 8 + c:9 + c])

        def x2_quad(q):
            jw = min(4, km - q * 4)
            for c in range(DC):
                pq = psA.tile([P, 512], F32, tag="ps_big", name=f"x2q_{q}_{c}")
                for j in range(jw):
                    nc.tensor.transpose(pq[:, j * P:(j + 1) * P],
                                        x2n[:, q * 4 + j, c * P:(c + 1) * P],
                                        ident[:])
                nc.scalar.copy(x2T[:, c, q * 512:q * 512 + jw * P],
                               pq[:, 0:jw * P])

        x1_quad(0)
        x2_quad(0)
        if km > 4:
            x2_quad(1)
        x1_quad(1)
        nc.sync.dma_start(out_r[:, 0:4, 0:D], x1n[:, 0:4, :])   # block 0 lo

        # s2 row: M=1 matmuls over x2T, then one batched set of col transposes
        s2row = rows.tile([1, vm], F32)
        for h, (off, w) in enumerate(rch):
            pr = psR.tile([1, 512], F32, tag="ps_row", name=f"s2r_{h}")
            for c in range(DC):
                nc.tensor.matmul(pr[0:1, 0:w], w2m[:, c:c + 1],
                                 x2T[:, c, off:off + w],
                                 start=(c == 0), stop=(c == DC - 1))
            nc.vector.tensor_copy(s2row[:, off:off + w], pr[0:1, 0:w])
        ps2 = psC.tile([P, km], F32, tag="ps_col")
        for u in range(km):
            nc.tensor.transpose(ps2[:, u:u + 1], s2row[0:1, u * P:(u + 1) * P],
                                ident[0:1, 0:1])
        es2 = const.tile([P, MT], F32)
        nc.scalar.activation(es2[:, 0:km], ps2[:], EXP)
        g2c = const.tile([P, MT], F32)
        nc.vector.tensor_mul(g2c[:, 0:km], es2[:, 0:km], valid2[:, 0:km])
        g2m = const.tile([P, MT], mm)
        if km < MT:
            nc.vector.memset(g2m[:, km:MT], 0.0)
        nc.vector.tensor_copy(g2m[:, 0:km], g2c[:, 0:km])

        # s1 row: same mechanism over x1w3T (recovers s1 = x1 @ w1 via u1)
        s1row = rows.tile([1, N], F32)
        for h, (off, w) in enumerate(nch):
            pr = psR.tile([1, 512], F32, tag="ps_row", name=f"s1r_{h}")
            for c in range(DC):
                nc.tensor.matmul(pr[0:1, 0:w], u1m[:, c:c + 1],
                                 x1w3T[:, c, off:off + w],
                                 start=(c == 0), stop=(c == DC - 1))
            nc.vector.tensor_copy(s1row[:, off:off + w], pr[0:1, 0:w])
        ps1 = psC.tile([P, NT], F32, tag="ps_col")
        for t in range(NT):
            nc.tensor.transpose(ps1[:, t:t + 1], s1row[0:1, t * P:(t + 1) * P],
                                ident[0:1, 0:1])
        es1 = const.tile([P, NT], F32)
        nc.scalar.activation(es1[:], ps1[:], EXP, scale=16.0)

        # g2-weighted x2 (rhs of U_row)
        x2g = big.tile([P, MT, D], mm)
        for u in range(km):
            nc.vector.tensor_scalar_mul(x2g[:, u, :], x2n[:, u, :],
                                        g2c[:, u:u + 1])

        # ---------------- E = exp(tri - 3) ----------------
        E = big.tile([P, NT, vm], mm)

        def e_tile(t):
            for h, (off, w) in enumerate(ech):
                pe = psA.tile([P, 512], F32, tag="ps_big", name=f"pe_{t}_{h}")
                cps = mm_pairs(DC)
                for i, (c, cw) in enumerate(cps):
                    nc.tensor.matmul(pe[:, 0:w],
                                     x1w3T[:, c:c + cw, t * P:(t + 1) * P],
                                     x2T[:, c:c + cw, off:off + w],
                                     start=(i == 0), stop=(i == len(cps) - 1),
                                     perf_mode=DR if (fp8 and cw == 2) else None)
                nc.scalar.activation(E[:, t, off:off + w], pe[:, 0:w], EXP,
                                     bias=ebias[:])

        # ---------------- ET = E^T (raw transposes) ----------------
        ET = big.tile([P, MT, N], mm)

        def et_quad(u, tq):
            # fp8 transpose mode requires output element step 2 in PSUM
            pq = psT.tile([P, 512, 2] if fp8 else [P, 512], mm, tag="ps_tr",
                          name=f"eq_{u}_{tq}")
            pqv = pq[:, :, 0] if fp8 else pq[:, :]
            for j in range(4):
                nc.tensor.transpose(pqv[:, j * P:(j + 1) * P],
                                    E[:, tq * 4 + j, u * P:(u + 1) * P],
                                    identm[:])
            if u % 2 == 0:
                nc.scalar.copy(ET[:, u, tq * 512:(tq + 1) * 512], pqv[:])
            else:
                nc.vector.tensor_copy(ET[:, u, tq * 512:(tq + 1) * 512], pqv[:])

        for t in range(4):
            e_tile(t)
        for u in range(km):
            et_quad(u, 0)
        for t in range(4, 8):
            e_tile(t)
        nc.sync.dma_start(out_r[:, 4:8, 0:D], x1n[:, 4:8, :])   # block 0 hi

        # g1 column + g1-weighted x1 (rhs of U_col)
        g1c = const.tile([P, NT], F32)
        nc.vector.tensor_mul(g1c[:], es1[:], valid1[:])
        g1m = const.tile([P, NT], mm)
        nc.vector.tensor_copy(g1m[:], g1c[:])
        x1g = big.tile([P, NT, D], mm)
        for k in range(kn):
            nc.vector.tensor_scalar_mul(x1g[:, k, :], x1n[:, k, :],
                                        g1c[:, k:k + 1])

        # den2[m] = sum_n g1[n] E[n,m]  (row matmuls, lhsT = g1 column pairs)
        d2row = rows.tile([1, vm], F32)
        for h, (off, w) in enumerate(rch):
            pr = psR.tile([1, 512], F32, tag="ps_row", name=f"d2r_{h}")
            for k in range(kn):
                nc.tensor.matmul(pr[0:1, 0:w], g1m[:, k:k + 1],
                                 E[:, k, off:off + w],
                                 start=(k == 0), stop=(k == kn - 1))
            nc.vector.tensor_copy(d2row[:, off:off + w], pr[0:1, 0:w])

        for u in range(km):
            et_quad(u, 1)

        pd2 = psC.tile([P, km], F32, tag="ps_col")
        for u in range(km):
            nc.tensor.transpose(pd2[:, u:u + 1], d2row[0:1, u * P:(u + 1) * P],
                                ident[0:1, 0:1])
        rden2 = const.tile([P, MT], F32)
        nc.vector.reciprocal(rden2[:, 0:km], pd2[:])
        qscale = const.tile([P, MT], F32)     # rden2 * g2: Q2C pre-gated for V
        nc.vector.tensor_mul(qscale[:, 0:km], rden2[:, 0:km], g2c[:, 0:km])

        # den1[n] = sum_m g2[m] ET[m,n]
        d1row = rows.tile([1, N], F32)
        for h, (off, w) in enumerate(nch):
            pr = psR.tile([1, 512], F32, tag="ps_row", name=f"d1r_{h}")
            for k in range(km):
                nc.tensor.matmul(pr[0:1, 0:w], g2m[:, k:k + 1],
                                 ET[:, k, off:off + w],
                                 start=(k == 0), stop=(k == km - 1))
            nc.vector.tensor_copy(d1row[:, off:off + w], pr[0:1, 0:w])
        pd1 = psC.tile([P, NT], F32, tag="ps_col")
        for t in range(NT):
            nc.tensor.transpose(pd1[:, t:t + 1], d1row[0:1, t * P:(t + 1) * P],
                                ident[0:1, 0:1])
        rden1 = const.tile([P, NT], F32)
        nc.vector.reciprocal(rden1[:], pd1[:])

        # x1 * rden1 — single-op block3 finals straight from PSUM
        x1r1 = big.tile([P, NT, D], F32)
        for t in range(NT):
            nc.vector.tensor_scalar_mul(x1r1[:, t, :], x1n[:, t, :],
                                        rden1[:, t:t + 1])

        # ---------------- U_col -> Q2C (pre-scaled by rden2*g2) ----------------
        Q2C = big.tile([P, MT, D], mm)
        for u in range(km):
            pu = psA.tile([P, 512], F32, tag="ps_big", name=f"pu_{u}")
            kps = mm_pairs(kn)
            for i, (k, kw) in enumerate(kps):
                nc.tensor.matmul(pu[:], E[:, k:k + kw, u * P:(u + 1) * P],
                                 x1g[:, k:k + kw, :],
                                 start=(i == 0), stop=(i == len(kps) - 1),
                                 perf_mode=DR if (fp8 and kw == 2) else None)
            nc.scalar.activation(Q2C[:, u, :], pu[:], COPY,
                                 scale=qscale[:, u:u + 1])

        # -------- U_row + V interleaved: blocks 1,2,3 spread evenly --------
        for t in range(NT):
            pr = psA.tile([P, 512], F32, tag="ps_big", name=f"pr_{t}")
            kps = mm_pairs(km)
            for i, (k, kw) in enumerate(kps):
                nc.tensor.matmul(pr[:], ET[:, k:k + kw, t * P:(t + 1) * P],
                                 x2g[:, k:k + kw, :],
                                 start=(i == 0), stop=(i == len(kps) - 1),
                                 perf_mode=DR if (fp8 and kw == 2) else None)
            combo = work.tile([P, 2 * D], F32, tag="ev", name=f"cb_{t}")
            nc.scalar.activation(combo[:, 0:D], pr[:], COPY,
                                 scale=rden1[:, t:t + 1])
            # block2 = x1*c2q on the otherwise-idle GpSimd engine (SBUF only)
            nc.gpsimd.tensor_mul(combo[:, D:2 * D], x1n[:, t, :], combo[:, 0:D])
            nc.sync.dma_start(out_r[:, t, D:3 * D], combo[:])

            pv = psA.tile([P, 512], F32, tag="ps_big", name=f"pv_{t}")
            for i, (k, kw) in enumerate(kps):
                nc.tensor.matmul(pv[:], ET[:, k:k + kw, t * P:(t + 1) * P],
                                 Q2C[:, k:k + kw, :],
                                 start=(i == 0), stop=(i == len(kps) - 1),
                                 perf_mode=DR if (fp8 and kw == 2) else None)
            b3 = work.tile([P, D], F32, tag="ev3", name=f"b3_{t}")
            nc.vector.tensor_mul(b3[:], x1r1[:, t, :], pv[:])
            nc.sync.dma_start(out_r[:, t, 3 * D:4 * D], b3[:])

    nc.compile()
    return nc


def _kept_tiles(mask):
    """Tiles (of 128) up to and including the last one with any valid row."""
    valid = ~mask.astype(bool)           # (b, L)
    any_valid = valid.reshape(valid.shape[0], -1, P).any(axis=2).any(axis=0)
    nz = np.nonzero(any_valid)[0]
    return int(nz[-1]) + 1 if len(nz) else 1


def _get_nc(kn, km):
    key = (kn, km)
    if key not in _CACHE:
        _CACHE[key] = _build(kn, km)
    return _CACHE[key]


def _run(inputs, trace=False, trace_cores=None):
    x1 = np.ascontiguousarray(np.asarray(inputs["x1"], dtype=np.float32))
    x2 = np.ascontiguousarray(np.asarray(inputs["x2"], dtype=np.float32))
    m1 = np.ascontiguousarray(np.asarray(inputs["x1_mask"]).astype(np.uint8))
    m2 = np.ascontiguousarray(np.asarray(inputs["x2_mask"]).astype(np.uint8))
    W = np.ascontiguousarray(np.asarray(inputs["W"], dtype=np.float32))
    nc = _get_nc(_kept_tiles(m1), _kept_tiles(m2))
    in_maps = [
        {"x1": x1[i], "x2": x2[i], "x1_mask": m1[i], "x2_mask": m2[i], "W": W}
        for i in range(N_CORES)
    ]
    res = run_bass_kernel_spmd(nc, in_maps, core_ids=list(range(N_CORES)),
                               trace=trace, trace_cores=trace_cores)
    out = np.stack([res.results[i]["out"] for i in range(N_CORES)], axis=0)
    return out.astype(np.float32), res


def kernel(x1, x1_mask, x2, x2_mask, W, bias=None, **_kw):
    # bias is mathematically irrelevant: a global additive constant cancels in
    # both softmaxes, and every output term is softmax-weighted.
    out, _ = _run({"x1": x1, "x1_mask": x1_mask, "x2": x2, "x2_mask": x2_mask,
                   "W": W})
    return out
